# revision 9
# baseline (speedup 1.0000x reference)
"""Trainium2 Bass kernel for: sigmoid(SimpleRNN(emb[x] @ Wxh + bh) @ Wd + bd).

Strategy (8-core data parallel, batch 4096 -> 512/core):
  - Rewrite emb[x] @ Wxh as (emb @ Wxh)[x]: each core projects 1/8 of the
    vocab (emb.T slice @ Wxh on PE, bf16), two AllGathers -> projected
    tables table_lo [31745, 128] / table_hi [18257, 128] bf16 in DRAM
    (last row of each is zero).
  - Gather per-token rows (256B) with dma_gather (int16 indices).  Every
    token is real in exactly one of lo/hi and points at the zero row in
    the other, so xp = g_lo + g_hi exactly (one DVE add, no select).
  - 80-step tanh recurrence: regular matmuls with the gathered tok-major
    tiles as stationary operand and identity as moving operand transpose
    xp into the fp32 PSUM bank; Whh matmul accumulates on top; ACT tanh
    (+bh) -> next h (bf16).  Final Wd matmul + sigmoid on ACT.
"""

import sys

if "/opt/trn_rl_repo" not in sys.path:
    sys.path.insert(0, "/opt/trn_rl_repo")

import numpy as np
import ml_dtypes

BF16 = ml_dtypes.bfloat16

VOCAB, EMB, SEQ, HID, BATCH = 50000, 300, 80, 128, 4096
NCORES = 8
BLOC = BATCH // NCORES        # 512 batch rows per core
VLOC = VOCAB // NCORES        # 6250 vocab rows per core
NJ = BLOC // 128              # 4 token tiles per timestep
CH_T = 8                      # timesteps per gather chunk
NCH = SEQ // CH_T             # 10 gather chunks
CHTOK = CH_T * BLOC           # 4096 tokens per chunk
GRP = 2048                    # vocab rows per phase-A group
SPLIT = 31744                 # lo/hi table split (31*1024: per-core shard 128-aligned)
LO_SH = SPLIT // NCORES       # 3968 lo rows per core
HI_SH = (VOCAB - SPLIT) // NCORES  # 2282 hi rows per core
LO_R = SPLIT + 1              # lo table rows (zero row at end)
HI_R = (VOCAB - SPLIT) + 1    # hi table rows (zero row at end)
KS = [(0, 128), (128, 128), (256, EMB - 256)]  # emb row chunks (K of matmul)

_CACHE = {}


def _phase_a(nc, tc, dt, embT, proj_lo, proj_hi, wxh_sb, rep):
    # local emb columns 0..LO_SH-1 -> proj_lo, LO_SH..VLOC-1 -> proj_hi
    GROUPS = [(0, 2048, 0, 0), (2048, LO_SH - 2048, 0, 2048),
              (LO_SH, 2048, 1, 0), (LO_SH + 2048, HI_SH - 2048, 1, 2048)]
    with tc.tile_pool(name=f"emba{rep}", bufs=2) as embp, \
         tc.tile_pool(name=f"prja{rep}", bufs=2) as projp, \
         tc.tile_pool(name=f"ppsum{rep}", bufs=4, space="PSUM") as ppsum:
        for (v0, w, is_hi, d0) in GROUPS:
            dst_t = proj_hi if is_hi else proj_lo
            ntile = (w + 127) // 128
            e_sb = []
            for ci, (k0, kn) in enumerate(KS):
                t = embp.tile([kn, GRP], dt.bfloat16, tag=f"e{ci}")
                nc.sync.dma_start(out=t[:, :w], in_=embT[k0:k0 + kn, v0:v0 + w])
                e_sb.append(t)
            ot = projp.tile([128, GRP // 128, HID], dt.bfloat16, tag="ot")
            for v in range(ntile):
                wv = min(128, w - v * 128)
                ps = ppsum.tile([128, HID], dt.float32, tag="pps")
                for ci, (k0, kn) in enumerate(KS):
                    nc.tensor.matmul(
                        out=ps[:wv, :],
                        lhsT=e_sb[ci][:, v * 128:v * 128 + wv],
                        rhs=wxh_sb[:kn, ci * 128:(ci + 1) * 128],
                        start=(ci == 0), stop=(ci == 2),
                    )
                nc.vector.tensor_copy(out=ot[:wv, v, :], in_=ps[:wv, :])
            if w % 128 == 0:
                dst = dst_t[d0:d0 + w, :].rearrange("(v p) f -> p v f", p=128)
                nc.sync.dma_start(out=dst, in_=ot[:, :w // 128, :])
            else:
                for v in range(ntile):
                    wv = min(128, w - v * 128)
                    nc.sync.dma_start(
                        out=dst_t[d0 + v * 128:d0 + v * 128 + wv, :],
                        in_=ot[:wv, v, :])


def _allgathers(nc, mybir, proj_lo, proj_hi, table_lo, table_hi):
    nc.gpsimd.collective_compute(
        "AllGather", mybir.AluOpType.bypass,
        replica_groups=[list(range(NCORES))],
        ins=[proj_lo[:]], outs=[table_lo[0:SPLIT, :]])
    nc.gpsimd.collective_compute(
        "AllGather", mybir.AluOpType.bypass,
        replica_groups=[list(range(NCORES))],
        ins=[proj_hi[:]], outs=[table_hi[0:VOCAB - SPLIT, :]])


def _phase_b(nc, tc, dt, AF, mybir, table_lo, table_hi, xlo_sb, xhi_sb,
             whh_sb, wd_sb, id_sb, bh_sb, bd_sb, out, rep):
    IW = CHTOK // 16  # idx columns per chunk in the 16-wrapped layout
    with tc.tile_pool(name=f"glo{rep}", bufs=2) as glop, \
         tc.tile_pool(name=f"ghi{rep}", bufs=2) as ghip, \
         tc.tile_pool(name=f"gsum{rep}", bufs=2) as gsump, \
         tc.tile_pool(name=f"h{rep}", bufs=2) as hp, \
         tc.tile_pool(name=f"rpsum{rep}", bufs=4, space="PSUM") as rpsum, \
         tc.tile_pool(name=f"outp{rep}", bufs=1) as outp:
        h_prev = None
        g_sum = None
        for t in range(SEQ):
            c, lt = divmod(t, CH_T)
            if lt == 0:
                g_lo = glop.tile([128, CH_T * NJ, HID], dt.bfloat16, tag="glo")
                g_hi = ghip.tile([128, CH_T * NJ, HID], dt.bfloat16, tag="ghi")
                nc.gpsimd.dma_gather(
                    out_ap=g_lo[:], in_ap=table_lo[:],
                    idxs_ap=xlo_sb[:, c * IW:(c + 1) * IW],
                    num_idxs=CHTOK, num_idxs_reg=CHTOK, elem_size=HID,
                    queue_num=0, single_packet=False)
                nc.gpsimd.dma_gather(
                    out_ap=g_hi[:], in_ap=table_hi[:],
                    idxs_ap=xhi_sb[:, c * IW:(c + 1) * IW],
                    num_idxs=CHTOK, num_idxs_reg=CHTOK, elem_size=HID,
                    queue_num=0, single_packet=False)
                g_sum = gsump.tile([128, CH_T * NJ, HID], dt.bfloat16, tag="gsum")
                nc.vector.tensor_tensor(
                    out=g_sum[:], in0=g_lo[:], in1=g_hi[:],
                    op=mybir.AluOpType.add)
            h_new = hp.tile([128, BLOC], dt.bfloat16, tag="h")
            for hh in range(2):  # half-batch phase-shifted chains
                ps = rpsum.tile([128, 256], dt.float32, tag="rps")
                for jj in range(2):
                    j = hh * 2 + jj
                    nc.tensor.matmul(
                        out=ps[:, jj * 128:(jj + 1) * 128],
                        lhsT=g_sum[:, lt * NJ + j, :],
                        rhs=id_sb,
                        start=(jj == 0),
                        stop=(t == 0 and jj == 1),
                        skip_group_check=True,
                    )
                if t > 0:
                    nc.tensor.matmul(
                        out=ps[:],
                        lhsT=whh_sb,
                        rhs=h_prev[:, hh * 256:(hh + 1) * 256],
                        start=False, stop=True,
                        skip_group_check=True,
                    )
                nc.scalar.activation(
                    out=h_new[:, hh * 256:(hh + 1) * 256],
                    in_=ps[:],
                    func=AF.Tanh,
                    bias=bh_sb,
                )
            h_prev = h_new

        # ---- output: sigmoid(h @ Wd + bd) ----
        ps_o = rpsum.tile([1, BLOC], dt.float32, tag="rpso")
        nc.tensor.matmul(out=ps_o[:], lhsT=wd_sb, rhs=h_prev[:],
                         start=True, stop=True)
        o_sb = outp.tile([1, BLOC], dt.float32)
        nc.scalar.activation(out=o_sb[:], in_=ps_o[:], func=AF.Sigmoid,
                             bias=bd_sb)
        nc.sync.dma_start(out=out[:], in_=o_sb[:])


def _build(repeat=1, phase_a=True, phase_b=True):
    import concourse.bacc as bacc
    import concourse.mybir as mybir
    from concourse.tile import TileContext

    dt = mybir.dt
    AF = mybir.ActivationFunctionType

    nc = bacc.Bacc("TRN2", target_bir_lowering=False, num_devices=NCORES)

    embT = nc.dram_tensor("embT", [EMB, VLOC], dt.bfloat16, kind="ExternalInput")
    wpack = nc.dram_tensor("wpack", [128, 3 * 128 + 128 + 1 + 128], dt.bfloat16,
                           kind="ExternalInput")
    fpack = nc.dram_tensor("fpack", [128, 2], dt.float32, kind="ExternalInput")
    xlo = nc.dram_tensor("xlo", [128, SEQ * BLOC // 16], dt.int16,
                         kind="ExternalInput")
    xhi = nc.dram_tensor("xhi", [128, SEQ * BLOC // 16], dt.int16,
                         kind="ExternalInput")
    out = nc.dram_tensor("out", [1, BLOC], dt.float32, kind="ExternalOutput")

    proj_lo = nc.dram_tensor("proj_lo", [LO_SH, HID], dt.bfloat16, kind="Internal")
    proj_hi = nc.dram_tensor("proj_hi", [HI_SH, HID], dt.bfloat16, kind="Internal")
    table_lo = nc.dram_tensor("table_lo", [LO_R, HID], dt.bfloat16, kind="Internal")
    table_hi = nc.dram_tensor("table_hi", [HI_R, HID], dt.bfloat16, kind="Internal")

    with TileContext(nc, num_cores=NCORES) as tc:
        with tc.tile_pool(name="const", bufs=1) as constp:
            wp_sb = constp.tile([128, 3 * 128 + 128 + 1 + 128], dt.bfloat16)
            nc.sync.dma_start(out=wp_sb[:], in_=wpack[:])
            fp_sb = constp.tile([128, 2], dt.float32)
            nc.sync.dma_start(out=fp_sb[:], in_=fpack[:])
            xlo_sb = constp.tile([128, SEQ * BLOC // 16], dt.int16)
            nc.sync.dma_start(out=xlo_sb[:], in_=xlo[:])
            xhi_sb = constp.tile([128, SEQ * BLOC // 16], dt.int16)
            nc.sync.dma_start(out=xhi_sb[:], in_=xhi[:])

            wxh_sb = wp_sb[:, 0:384]
            whh_sb = wp_sb[:, 384:512]
            wd_sb = wp_sb[:, 512:513]
            id_sb = wp_sb[:, 513:641]
            bh_sb = fp_sb[:, 0:1]
            bd_sb = fp_sb[0:1, 1:2]

            # zero rows at the end of each table
            zrow = constp.tile([1, HID], dt.bfloat16)
            nc.gpsimd.memset(zrow[:], 0.0)
            nc.sync.dma_start(out=table_lo[LO_R - 1:LO_R, :], in_=zrow[:])
            nc.sync.dma_start(out=table_hi[HI_R - 1:HI_R, :], in_=zrow[:])

            for rep in range(repeat):
                if phase_a:
                    _phase_a(nc, tc, dt, embT, proj_lo, proj_hi, wxh_sb, rep)
                    _allgathers(nc, mybir, proj_lo, proj_hi, table_lo, table_hi)
                if phase_b:
                    _phase_b(nc, tc, dt, AF, mybir, table_lo, table_hi,
                             xlo_sb, xhi_sb, whh_sb, wd_sb, id_sb, bh_sb,
                             bd_sb, out, rep)

    nc.compile()
    return nc


def _wrap16(idx_flat):
    """[N] int16 token-order indices -> [128, N/16] wrapped+replicated layout.

    dma_gather consumes index i from partition i%16, column i//16 (the 16-row
    block replicated across the 8 Q7 cores' partition groups)."""
    n = idx_flat.shape[0]
    arr = idx_flat.reshape(n // 16, 16).T.astype(np.int16)  # [16, n/16]
    return np.ascontiguousarray(np.tile(arr, (8, 1)))


def _prep_inputs(x, emb, Wxh, Whh, bh, Wd, bd):
    x = np.asarray(x)
    emb = np.asarray(emb, np.float32)
    Wxh = np.asarray(Wxh, np.float32)
    Whh = np.asarray(Whh, np.float32)
    bh = np.asarray(bh, np.float32)
    Wd = np.asarray(Wd, np.float32)
    bd = np.asarray(bd, np.float32)

    wpack = np.zeros((128, 3 * 128 + 128 + 1 + 128), BF16)
    for ci, (k0, kn) in enumerate(KS):
        wpack[:kn, ci * 128:ci * 128 + 128] = Wxh[k0:k0 + kn, :].astype(BF16)
    wpack[:, 384:512] = Whh.astype(BF16)
    wpack[:, 512] = Wd[:, 0].astype(BF16)
    wpack[:, 513:641] = np.eye(128, dtype=BF16)

    fpack = np.zeros((128, 2), np.float32)
    fpack[:, 0] = bh
    fpack[0, 1] = bd[0]

    embT = np.ascontiguousarray(emb.T.astype(BF16))  # [300, 50000]

    in_maps = []
    for c in range(NCORES):
        xc = np.asarray(x[c * BLOC:(c + 1) * BLOC, :], np.int64)  # [512, 80]
        # token order: t-major, then column b' = j*128+p
        rows = xc.T.reshape(-1)            # [80*512] vocab ids
        lo = np.where(rows < SPLIT, rows, LO_R - 1)
        hi = np.where(rows >= SPLIT, rows - SPLIT, HI_R - 1)
        emb_cols = np.concatenate([
            np.arange(c * LO_SH, (c + 1) * LO_SH),
            np.arange(SPLIT + c * HI_SH, SPLIT + (c + 1) * HI_SH)])
        in_maps.append({
            "embT": np.ascontiguousarray(embT[:, emb_cols]),
            "wpack": wpack,
            "fpack": fpack,
            "xlo": _wrap16(lo),
            "xhi": _wrap16(hi),
        })
    return in_maps


def kernel(x, emb, Wxh, Whh, bh, Wd, bd):
    from concourse import bass_utils

    if "nc" not in _CACHE:
        _CACHE["nc"] = _build()
    nc = _CACHE["nc"]

    in_maps = _prep_inputs(x, emb, Wxh, Whh, bh, Wd, bd)
    res = bass_utils.run_bass_kernel_spmd(
        nc, in_maps, core_ids=list(range(NCORES)),
        trace=bool(_CACHE.get("trace")),
    )
    _CACHE["last_result"] = res
    out = np.concatenate([res.results[c]["out"][0] for c in range(NCORES)])
    return out.reshape(BATCH, 1).astype(np.float32)


# revision 12
# speedup vs baseline: 1.5652x; 1.5652x over previous
"""Trainium2 Bass kernel for: sigmoid(SimpleRNN(emb[x] @ Wxh + bh) @ Wd + bd).

Strategy (8-core data parallel, batch 4096 -> 512/core):
  - Rewrite emb[x] @ Wxh as (emb @ Wxh)[x]: each core projects 1/8 of the
    vocab (emb.T slice @ Wxh on PE, bf16), two AllGathers -> projected
    tables table_lo [31745, 128] / table_hi [18257, 128] bf16 in DRAM
    (last row of each is zero).
  - Gather per-token rows (256B) with dma_gather (int16 indices).  Every
    token is real in exactly one of lo/hi and points at the zero row in
    the other, so xp = g_lo + g_hi exactly (one DVE add, no select).
  - 80-step tanh recurrence: regular matmuls with the gathered tok-major
    tiles as stationary operand and identity as moving operand transpose
    xp into the fp32 PSUM bank; Whh matmul accumulates on top; ACT tanh
    (+bh) -> next h (bf16).  Final Wd matmul + sigmoid on ACT.
"""

import sys

if "/opt/trn_rl_repo" not in sys.path:
    sys.path.insert(0, "/opt/trn_rl_repo")

import numpy as np
import ml_dtypes

BF16 = ml_dtypes.bfloat16

VOCAB, EMB, SEQ, HID, BATCH = 50000, 300, 80, 128, 4096
NCORES = 8
BLOC = BATCH // NCORES        # 512 batch rows per core
VLOC = VOCAB // NCORES        # 6250 vocab rows per core
NJ = BLOC // 128              # 4 token tiles per timestep
CH_T = 8                      # timesteps per gather chunk
NCH = SEQ // CH_T             # 10 gather chunks
CHTOK = CH_T * BLOC           # 4096 tokens per chunk
GRP = 2048                    # vocab rows per phase-A group
KS = [(0, 128), (128, 128), (256, EMB - 256)]  # emb row chunks (K of matmul)

_CACHE = {}


def _phase_a(nc, tc, dt, embT, proj_in, wxh_sb, rep):
    GROUPS = [(v0, min(GRP, VLOC - v0)) for v0 in range(0, VLOC, GRP)]
    with tc.tile_pool(name=f"emba{rep}", bufs=2) as embp, \
         tc.tile_pool(name=f"prja{rep}", bufs=2) as projp, \
         tc.tile_pool(name=f"ppsum{rep}", bufs=4, space="PSUM") as ppsum:
        for (v0, w) in GROUPS:
            dst_t, d0 = proj_in, v0
            ntile = (w + 127) // 128
            e_sb = []
            for ci, (k0, kn) in enumerate(KS):
                t = embp.tile([kn, GRP], dt.bfloat16, tag=f"e{ci}")
                nc.sync.dma_start(out=t[:, :w], in_=embT[k0:k0 + kn, v0:v0 + w])
                e_sb.append(t)
            ot = projp.tile([128, GRP // 128, HID], dt.bfloat16, tag="ot")
            for v in range(ntile):
                wv = min(128, w - v * 128)
                ps = ppsum.tile([128, HID], dt.float32, tag="pps")
                for ci, (k0, kn) in enumerate(KS):
                    nc.tensor.matmul(
                        out=ps[:wv, :],
                        lhsT=e_sb[ci][:, v * 128:v * 128 + wv],
                        rhs=wxh_sb[:kn, ci * 128:(ci + 1) * 128],
                        start=(ci == 0), stop=(ci == 2),
                    )
                nc.vector.tensor_copy(out=ot[:wv, v, :], in_=ps[:wv, :])
            if w % 128 == 0:
                dst = dst_t[d0:d0 + w, :].rearrange("(v p) f -> p v f", p=128)
                nc.sync.dma_start(out=dst, in_=ot[:, :w // 128, :])
            else:
                for v in range(ntile):
                    wv = min(128, w - v * 128)
                    nc.sync.dma_start(
                        out=dst_t[d0 + v * 128:d0 + v * 128 + wv, :],
                        in_=ot[:wv, v, :])


def _allgathers(nc, mybir, proj_in, table):
    nc.gpsimd.collective_compute(
        "AllGather", mybir.AluOpType.bypass,
        replica_groups=[list(range(NCORES))],
        ins=[proj_in[:]], outs=[table[:]])


def _phase_b(nc, tc, dt, AF, mybir, table, xi_sb, m_sb,
             whh_sb, wd_sb, id_sb, bh_sb, bd_sb, out, rep, nsplit=2):
    IW = CHTOK // 16  # idx columns per chunk in the 16-wrapped layout
    BW = BLOC // nsplit           # batch columns per split
    TJ = BW // 128                # token tiles per split
    with tc.tile_pool(name=f"gg{rep}", bufs=3) as ggp, \
         tc.tile_pool(name=f"h{rep}", bufs=2) as hp, \
         tc.tile_pool(name=f"rpsum{rep}", bufs=4, space="PSUM") as rpsum, \
         tc.tile_pool(name=f"outp{rep}", bufs=1) as outp:
        h_prev = None
        g_sum = None
        for t in range(SEQ):
            c, lt = divmod(t, CH_T)
            if lt == 0:
                g_sum = ggp.tile([128, CH_T * NJ, 2 * HID], dt.bfloat16, tag="gg")
                nc.gpsimd.dma_gather(
                    out_ap=g_sum[:], in_ap=table[:],
                    idxs_ap=xi_sb[:, c * IW:(c + 1) * IW],
                    num_idxs=CHTOK, num_idxs_reg=CHTOK, elem_size=2 * HID,
                    queue_num=0, single_packet=False)
                mb = m_sb[:, c * CH_T * NJ:(c + 1) * CH_T * NJ]
                mb = mb.rearrange("p (k one) -> p k one", one=1)
                mb = mb.broadcast_to([128, CH_T * NJ, HID])
                nc.vector.copy_predicated(
                    out=g_sum[:, :, 0:HID], mask=mb,
                    data=g_sum[:, :, HID:2 * HID])
            h_new = hp.tile([128, BLOC], dt.bfloat16, tag="h")
            for hh in range(nsplit):  # sub-batch phase-shifted chains
                ps = rpsum.tile([128, BW], dt.float32, tag="rps")
                for jj in range(TJ):
                    j = hh * TJ + jj
                    nc.tensor.matmul(
                        out=ps[:, jj * 128:(jj + 1) * 128],
                        lhsT=g_sum[:, lt * NJ + j, 0:HID],
                        rhs=id_sb,
                        start=(jj == 0),
                        stop=(t == 0 and jj == TJ - 1),
                        skip_group_check=True,
                    )
                if t > 0:
                    nc.tensor.matmul(
                        out=ps[:],
                        lhsT=whh_sb,
                        rhs=h_prev[:, hh * BW:(hh + 1) * BW],
                        start=False, stop=True,
                        skip_group_check=True,
                    )
                nc.scalar.activation(
                    out=h_new[:, hh * BW:(hh + 1) * BW],
                    in_=ps[:],
                    func=AF.Tanh,
                    bias=bh_sb,
                )
            h_prev = h_new

        # ---- output: sigmoid(h @ Wd + bd) ----
        ps_o = rpsum.tile([1, BLOC], dt.float32, tag="rpso")
        nc.tensor.matmul(out=ps_o[:], lhsT=wd_sb, rhs=h_prev[:],
                         start=True, stop=True)
        o_sb = outp.tile([1, BLOC], dt.float32)
        nc.scalar.activation(out=o_sb[:], in_=ps_o[:], func=AF.Sigmoid,
                             bias=bd_sb)
        nc.sync.dma_start(out=out[:], in_=o_sb[:])


def _build(repeat=1, phase_a=True, phase_b=True, nsplit=2, ch_t=CH_T):
    import concourse.bacc as bacc
    import concourse.mybir as mybir
    from concourse.tile import TileContext

    dt = mybir.dt
    AF = mybir.ActivationFunctionType

    nc = bacc.Bacc("TRN2", target_bir_lowering=False, num_devices=NCORES)

    embT = nc.dram_tensor("embT", [EMB, VLOC], dt.bfloat16, kind="ExternalInput")
    wpack = nc.dram_tensor("wpack", [128, 3 * 128 + 128 + 1 + 128], dt.bfloat16,
                           kind="ExternalInput")
    fpack = nc.dram_tensor("fpack", [128, 2], dt.float32, kind="ExternalInput")
    xidx = nc.dram_tensor("xidx", [128, SEQ * BLOC // 16], dt.int16,
                          kind="ExternalInput")
    xmask = nc.dram_tensor("xmask", [128, SEQ * NJ], dt.int8,
                           kind="ExternalInput")
    out = nc.dram_tensor("out", [1, BLOC], dt.float32, kind="ExternalOutput")

    proj_in = nc.dram_tensor("proj_in", [VLOC, HID], dt.bfloat16, kind="Internal")
    table = nc.dram_tensor("table", [VOCAB // 2, 2 * HID], dt.bfloat16,
                           kind="Internal")

    with TileContext(nc, num_cores=NCORES) as tc:
        with tc.tile_pool(name="const", bufs=1) as constp:
            wp_sb = constp.tile([128, 3 * 128 + 128 + 1 + 128], dt.bfloat16)
            nc.sync.dma_start(out=wp_sb[:], in_=wpack[:])
            fp_sb = constp.tile([128, 2], dt.float32)
            nc.sync.dma_start(out=fp_sb[:], in_=fpack[:])
            xi_sb = constp.tile([128, SEQ * BLOC // 16], dt.int16)
            nc.sync.dma_start(out=xi_sb[:], in_=xidx[:])
            m_sb = constp.tile([128, SEQ * NJ], dt.int8)
            nc.sync.dma_start(out=m_sb[:], in_=xmask[:])

            wxh_sb = wp_sb[:, 0:384]
            whh_sb = wp_sb[:, 384:512]
            wd_sb = wp_sb[:, 512:513]
            id_sb = wp_sb[:, 513:641]
            bh_sb = fp_sb[:, 0:1]
            bd_sb = fp_sb[0:1, 1:2]

            for rep in range(repeat):
                if phase_a:
                    _phase_a(nc, tc, dt, embT, proj_in, wxh_sb, rep)
                    _allgathers(nc, mybir, proj_in, table)
                if phase_b:
                    _phase_b(nc, tc, dt, AF, mybir, table,
                             xi_sb, m_sb, whh_sb, wd_sb, id_sb, bh_sb,
                             bd_sb, out, rep, nsplit=nsplit)

    nc.compile()
    return nc


def _wrap16(idx_flat):
    """[N] int16 token-order indices -> [128, N/16] wrapped+replicated layout.

    dma_gather consumes index i from partition i%16, column i//16 (the 16-row
    block replicated across the 8 Q7 cores' partition groups)."""
    n = idx_flat.shape[0]
    arr = idx_flat.reshape(n // 16, 16).T.astype(np.int16)  # [16, n/16]
    return np.ascontiguousarray(np.tile(arr, (8, 1)))


def _prep_inputs(x, emb, Wxh, Whh, bh, Wd, bd):
    x = np.asarray(x)
    emb = np.asarray(emb, np.float32)
    Wxh = np.asarray(Wxh, np.float32)
    Whh = np.asarray(Whh, np.float32)
    bh = np.asarray(bh, np.float32)
    Wd = np.asarray(Wd, np.float32)
    bd = np.asarray(bd, np.float32)

    wpack = np.zeros((128, 3 * 128 + 128 + 1 + 128), BF16)
    for ci, (k0, kn) in enumerate(KS):
        wpack[:kn, ci * 128:ci * 128 + 128] = Wxh[k0:k0 + kn, :].astype(BF16)
    wpack[:, 384:512] = Whh.astype(BF16)
    wpack[:, 512] = Wd[:, 0].astype(BF16)
    wpack[:, 513:641] = np.eye(128, dtype=BF16)

    fpack = np.zeros((128, 2), np.float32)
    fpack[:, 0] = bh
    fpack[0, 1] = bd[0]

    embT = np.ascontiguousarray(emb.T.astype(BF16))  # [300, 50000]

    in_maps = []
    for c in range(NCORES):
        xc = np.asarray(x[c * BLOC:(c + 1) * BLOC, :], np.int64)  # [512, 80]
        # token order: t-major, then column b' = j*128+p
        rows = xc.T.reshape(-1)            # [80*512] vocab ids
        mask = (rows & 1).astype(np.int8).reshape(SEQ * NJ, 128).T
        in_maps.append({
            "embT": np.ascontiguousarray(embT[:, c * VLOC:(c + 1) * VLOC]),
            "wpack": wpack,
            "fpack": fpack,
            "xidx": _wrap16(rows >> 1),
            "xmask": np.ascontiguousarray(mask),
        })
    return in_maps


def kernel(x, emb, Wxh, Whh, bh, Wd, bd):
    from concourse import bass_utils

    if "nc" not in _CACHE:
        _CACHE["nc"] = _build()
    nc = _CACHE["nc"]

    in_maps = _prep_inputs(x, emb, Wxh, Whh, bh, Wd, bd)
    res = bass_utils.run_bass_kernel_spmd(
        nc, in_maps, core_ids=list(range(NCORES)),
        trace=bool(_CACHE.get("trace")),
    )
    _CACHE["last_result"] = res
    out = np.concatenate([res.results[c]["out"][0] for c in range(NCORES)])
    return out.reshape(BATCH, 1).astype(np.float32)


# revision 13
# speedup vs baseline: 5.3003x; 3.3863x over previous
"""Trainium2 Bass kernel for: sigmoid(SimpleRNN(emb[x] @ Wxh + bh) @ Wd + bd).

Strategy (8-core data parallel, batch 4096 -> 512/core):
  - Rewrite emb[x] @ Wxh as (emb @ Wxh)[x]: each core projects 1/8 of the
    vocab (emb.T slice @ Wxh on PE, bf16), two AllGathers -> projected
    tables table_lo [31745, 128] / table_hi [18257, 128] bf16 in DRAM
    (last row of each is zero).
  - Gather per-token rows (256B) with dma_gather (int16 indices).  Every
    token is real in exactly one of lo/hi and points at the zero row in
    the other, so xp = g_lo + g_hi exactly (one DVE add, no select).
  - 80-step tanh recurrence: regular matmuls with the gathered tok-major
    tiles as stationary operand and identity as moving operand transpose
    xp into the fp32 PSUM bank; Whh matmul accumulates on top; ACT tanh
    (+bh) -> next h (bf16).  Final Wd matmul + sigmoid on ACT.
"""

import sys

if "/opt/trn_rl_repo" not in sys.path:
    sys.path.insert(0, "/opt/trn_rl_repo")

import numpy as np
import ml_dtypes

BF16 = ml_dtypes.bfloat16

VOCAB, EMB, SEQ, HID, BATCH = 50000, 300, 80, 128, 4096
NCORES = 8
BLOC = BATCH // NCORES        # 512 batch rows per core
VLOC = VOCAB // NCORES        # 6250 vocab rows per core
NJ = BLOC // 128              # 4 token tiles per timestep
CH_T = 8                      # timesteps per gather chunk
NCH = SEQ // CH_T             # 10 gather chunks
CHTOK = CH_T * BLOC           # 4096 tokens per chunk
GRP = 2048                    # vocab rows per phase-A group
KS = [(0, 128), (128, 128), (256, EMB - 256)]  # emb row chunks (K of matmul)

_CACHE = {}


def _phase_a(nc, tc, dt, embT, proj_in, wxh_sb, rep):
    GROUPS = [(v0, min(GRP, VLOC - v0)) for v0 in range(0, VLOC, GRP)]
    with tc.tile_pool(name=f"emba{rep}", bufs=2) as embp, \
         tc.tile_pool(name=f"prja{rep}", bufs=2) as projp, \
         tc.tile_pool(name=f"ppsum{rep}", bufs=4, space="PSUM") as ppsum:
        for (v0, w) in GROUPS:
            dst_t, d0 = proj_in, v0
            ntile = (w + 127) // 128
            e_sb = []
            for ci, (k0, kn) in enumerate(KS):
                t = embp.tile([kn, GRP], dt.bfloat16, tag=f"e{ci}")
                nc.sync.dma_start(out=t[:, :w], in_=embT[k0:k0 + kn, v0:v0 + w])
                e_sb.append(t)
            ot = projp.tile([128, GRP // 128, HID], dt.bfloat16, tag="ot")
            for v in range(ntile):
                wv = min(128, w - v * 128)
                ps = ppsum.tile([128, HID], dt.float32, tag="pps")
                for ci, (k0, kn) in enumerate(KS):
                    nc.tensor.matmul(
                        out=ps[:wv, :],
                        lhsT=e_sb[ci][:, v * 128:v * 128 + wv],
                        rhs=wxh_sb[:kn, ci * 128:(ci + 1) * 128],
                        start=(ci == 0), stop=(ci == 2),
                    )
                nc.vector.tensor_copy(out=ot[:wv, v, :], in_=ps[:wv, :])
            if w % 128 == 0:
                dst = dst_t[d0:d0 + w, :].rearrange("(v p) f -> p v f", p=128)
                nc.sync.dma_start(out=dst, in_=ot[:, :w // 128, :])
            else:
                for v in range(ntile):
                    wv = min(128, w - v * 128)
                    nc.sync.dma_start(
                        out=dst_t[d0 + v * 128:d0 + v * 128 + wv, :],
                        in_=ot[:wv, v, :])


def _allgathers(nc, mybir, proj_in, table):
    nc.gpsimd.collective_compute(
        "AllGather", mybir.AluOpType.bypass,
        replica_groups=[list(range(NCORES))],
        ins=[proj_in[:]], outs=[table[:]])


def _phase_b(nc, tc, dt, AF, mybir, table, xi_sb, m_sb,
             whh_sb, wd_sb, id_sb, bh_sb, bd_sb, out, rep, nsplit=2):
    IW = CHTOK // 16  # idx columns per chunk in the 16-wrapped layout
    BW = BLOC // nsplit           # batch columns per split
    TJ = BW // 128                # token tiles per split
    with tc.tile_pool(name=f"gg{rep}", bufs=3) as ggp, \
         tc.tile_pool(name=f"h{rep}", bufs=2) as hp, \
         tc.tile_pool(name=f"rpsum{rep}", bufs=4, space="PSUM") as rpsum, \
         tc.tile_pool(name=f"outp{rep}", bufs=1) as outp:
        h_prev = None
        g_sum = None
        for t in range(SEQ):
            c, lt = divmod(t, CH_T)
            if lt == 0:
                g_sum = ggp.tile([128, CH_T * NJ, 2 * HID], dt.bfloat16, tag="gg")
                nc.gpsimd.dma_gather(
                    out_ap=g_sum[:], in_ap=table[:],
                    idxs_ap=xi_sb[:, c * IW:(c + 1) * IW],
                    num_idxs=CHTOK, num_idxs_reg=CHTOK, elem_size=2 * HID,
                    queue_num=0, single_packet=False)
                mb = m_sb[:, c * CH_T * NJ:(c + 1) * CH_T * NJ]
                mb = mb.rearrange("p (k one) -> p k one", one=1)
                mb = mb.broadcast_to([128, CH_T * NJ, HID])
                nc.vector.copy_predicated(
                    out=g_sum[:, :, 0:HID], mask=mb,
                    data=g_sum[:, :, HID:2 * HID])
            h_new = hp.tile([128, BLOC], dt.bfloat16, tag="h")
            for hh in range(nsplit):  # sub-batch phase-shifted chains
                ps = rpsum.tile([128, BW], dt.float32, tag="rps")
                for jj in range(TJ):
                    j = hh * TJ + jj
                    nc.tensor.matmul(
                        out=ps[:, jj * 128:(jj + 1) * 128],
                        lhsT=g_sum[:, lt * NJ + j, 0:HID],
                        rhs=id_sb,
                        start=(jj == 0),
                        stop=(t == 0 and jj == TJ - 1),
                        skip_group_check=True,
                    )
                if t > 0:
                    nc.tensor.matmul(
                        out=ps[:],
                        lhsT=whh_sb,
                        rhs=h_prev[:, hh * BW:(hh + 1) * BW],
                        start=False, stop=True,
                        skip_group_check=True,
                    )
                nc.scalar.activation(
                    out=h_new[:, hh * BW:(hh + 1) * BW],
                    in_=ps[:],
                    func=AF.Tanh,
                    bias=bh_sb,
                )
            h_prev = h_new

        # ---- output: sigmoid(h @ Wd + bd) ----
        ps_o = rpsum.tile([1, BLOC], dt.float32, tag="rpso")
        nc.tensor.matmul(out=ps_o[:], lhsT=wd_sb, rhs=h_prev[:],
                         start=True, stop=True)
        o_sb = outp.tile([1, BLOC], dt.float32)
        nc.scalar.activation(out=o_sb[:], in_=ps_o[:], func=AF.Sigmoid,
                             bias=bd_sb)
        nc.sync.dma_start(out=out[:], in_=o_sb[:])


def _build(repeat=1, phase_a=True, phase_b=True, nsplit=2, do_ag=True):
    import concourse.bacc as bacc
    import concourse.mybir as mybir
    from concourse.tile import TileContext

    dt = mybir.dt
    AF = mybir.ActivationFunctionType

    nc = bacc.Bacc("TRN2", target_bir_lowering=False, num_devices=NCORES)

    embT = nc.dram_tensor("embT", [EMB, VLOC], dt.bfloat16, kind="ExternalInput")
    wpack = nc.dram_tensor("wpack", [128, 3 * 128 + 128 + 1 + 128], dt.bfloat16,
                           kind="ExternalInput")
    fpack = nc.dram_tensor("fpack", [128, 2], dt.float32, kind="ExternalInput")
    xidx = nc.dram_tensor("xidx", [128, SEQ * BLOC // 16], dt.int16,
                          kind="ExternalInput")
    xmask = nc.dram_tensor("xmask", [128, SEQ * NJ], dt.int8,
                           kind="ExternalInput")
    out = nc.dram_tensor("out", [1, BLOC], dt.float32, kind="ExternalOutput")

    proj_in = nc.dram_tensor("proj_in", [VLOC, HID], dt.bfloat16, kind="Internal")
    table = nc.dram_tensor("table", [VOCAB // 2, 2 * HID], dt.bfloat16,
                           kind="Internal")

    with TileContext(nc, num_cores=NCORES) as tc:
        with tc.tile_pool(name="const", bufs=1) as constp:
            wp_sb = constp.tile([128, 3 * 128 + 128 + 1 + 128], dt.bfloat16)
            nc.sync.dma_start(out=wp_sb[:], in_=wpack[:])
            fp_sb = constp.tile([128, 2], dt.float32)
            nc.sync.dma_start(out=fp_sb[:], in_=fpack[:])
            xi_sb = constp.tile([128, SEQ * BLOC // 16], dt.int16)
            nc.sync.dma_start(out=xi_sb[:], in_=xidx[:])
            m_sb = constp.tile([128, SEQ * NJ], dt.int8)
            nc.sync.dma_start(out=m_sb[:], in_=xmask[:])

            wxh_sb = wp_sb[:, 0:384]
            whh_sb = wp_sb[:, 384:512]
            wd_sb = wp_sb[:, 512:513]
            id_sb = wp_sb[:, 513:641]
            bh_sb = fp_sb[:, 0:1]
            bd_sb = fp_sb[0:1, 1:2]

            for rep in range(repeat):
                if phase_a:
                    _phase_a(nc, tc, dt, embT, proj_in, wxh_sb, rep)
                    if do_ag:
                        _allgathers(nc, mybir, proj_in, table)
                if phase_b:
                    _phase_b(nc, tc, dt, AF, mybir, table,
                             xi_sb, m_sb, whh_sb, wd_sb, id_sb, bh_sb,
                             bd_sb, out, rep, nsplit=nsplit)

    nc.compile()
    return nc


def _wrap16(idx_flat):
    """[N] int16 token-order indices -> [128, N/16] wrapped+replicated layout.

    dma_gather consumes index i from partition i%16, column i//16 (the 16-row
    block replicated across the 8 Q7 cores' partition groups)."""
    n = idx_flat.shape[0]
    arr = idx_flat.reshape(n // 16, 16).T.astype(np.int16)  # [16, n/16]
    return np.ascontiguousarray(np.tile(arr, (8, 1)))


def _prep_inputs(x, emb, Wxh, Whh, bh, Wd, bd):
    x = np.asarray(x)
    emb = np.asarray(emb, np.float32)
    Wxh = np.asarray(Wxh, np.float32)
    Whh = np.asarray(Whh, np.float32)
    bh = np.asarray(bh, np.float32)
    Wd = np.asarray(Wd, np.float32)
    bd = np.asarray(bd, np.float32)

    wpack = np.zeros((128, 3 * 128 + 128 + 1 + 128), BF16)
    for ci, (k0, kn) in enumerate(KS):
        wpack[:kn, ci * 128:ci * 128 + 128] = Wxh[k0:k0 + kn, :].astype(BF16)
    wpack[:, 384:512] = Whh.astype(BF16)
    wpack[:, 512] = Wd[:, 0].astype(BF16)
    wpack[:, 513:641] = np.eye(128, dtype=BF16)

    fpack = np.zeros((128, 2), np.float32)
    fpack[:, 0] = bh
    fpack[0, 1] = bd[0]

    embT = np.ascontiguousarray(emb.T.astype(BF16))  # [300, 50000]

    in_maps = []
    for c in range(NCORES):
        xc = np.asarray(x[c * BLOC:(c + 1) * BLOC, :], np.int64)  # [512, 80]
        # token order: t-major, then column b' = j*128+p
        rows = xc.T.reshape(-1)            # [80*512] vocab ids
        mask = (rows & 1).astype(np.int8).reshape(SEQ * NJ, 128).T
        in_maps.append({
            "embT": np.ascontiguousarray(embT[:, c * VLOC:(c + 1) * VLOC]),
            "wpack": wpack,
            "fpack": fpack,
            "xidx": _wrap16(rows >> 1),
            "xmask": np.ascontiguousarray(mask),
        })
    return in_maps


def kernel(x, emb, Wxh, Whh, bh, Wd, bd):
    from concourse import bass_utils

    if "nc" not in _CACHE:
        _CACHE["nc"] = _build()
    nc = _CACHE["nc"]

    in_maps = _prep_inputs(x, emb, Wxh, Whh, bh, Wd, bd)
    res = bass_utils.run_bass_kernel_spmd(
        nc, in_maps, core_ids=list(range(NCORES)),
        trace=bool(_CACHE.get("trace")),
    )
    _CACHE["last_result"] = res
    out = np.concatenate([res.results[c]["out"][0] for c in range(NCORES)])
    return out.reshape(BATCH, 1).astype(np.float32)
